# revision 18
# baseline (speedup 1.0000x reference)
"""Trainium2 Bass kernel for nn_ActionDecoder (MoE-routed 2-layer GELU MLP).

Problem: per batch row b (2048 rows x 16 timesteps), route through the
embodiment_ids[b]-th expert MLP: out = GELU(x @ W1[e] + b1[e]) @ W2[e] + b2[e].
x: [2048, 16, 512] f32, W1: [4, 512, 1024], W2: [4, 1024, 28].

Strategy (expert-parallel): host sorts batch rows by embodiment, gives each of
the 8 cores one expert (2 cores per expert, half the expert's rows each). Each
core runs a dense 2-layer MLP over its tokens with its own expert's weights
(weights are per-core *data*, so one SPMD program serves all cores). Activations
are fed transposed ([d, tok]) so both matmuls keep weights stationary.

Mixed-precision: h-chunks 0..DR_CHUNKS-1 contract dims 0..255 via ONE fp8e4
DoubleRow matmul (256 dims/instr at the same ~216ns stream rate as a 128-dim
fp16 matmul) plus two fp16 matmuls for dims 256..511; the remaining h-chunks
use four fp16 matmuls. All accumulate in one PSUM group per h-chunk; the
global scales (sx, sw powers of two sized for e4m3's +-240 range) are folded
into the GELU activation's `scale` immediate.

Measured error model (HW matches exact host emulation of the quantized
arithmetic to 2.4e-4, so error == quantization noise): DR on 8/8 chunks ->
rel err 2.59e-2 (gate 2e-2); err scales ~sqrt(DR_CHUNKS/8): 4/8 measures
1.9347e-2 (PASS, deterministic across runs). 5/8 would be ~2.16e-2 (FAIL).

HW facts learned by probing (probe_dr*.py, probe_timing.py):
- DoubleRow layout: stationary [K=128p, 2, M], moving [K=128p, 2, N], pair
  dim MIDDLE (AP dim1 must have Num=2, step%16==0 - BIR verifier). Semantics
  out = sum_i w[:,i].T @ x[:,i]. Mixing DR + fp16 matmuls in one PSUM
  accumulation group is exact.
- DO NOT hold two accumulation groups open with interleaved DR starts
  (DR->psA, DR->psB, f16s->psA, f16s->psB corrupts results: rel err 0.41).
  Keep one group open at a time: DR, f16, f16, close; next group.
- Stream rates (512-col moving, warm): fp16 215.8ns, fp8e4-plain 215.8,
  fp8e4-DoublePixel 215.8 (mode is a silent no-op), fp8e4-DR back-to-back
  226.7ns; in the DR,f16,f16 chunk pattern DR is ~245ns effective (its
  256-col LDWEIGHTS hides under the two preceding f16 streams).
  => DR chunk = ~681ns vs clean fp16 chunk = ~872ns per 512-token tile.
- e3m4 (float8e3) + DoubleRow is hard-rejected by walrus birverifier
  (inst_visitor.cpp:3011), both via bass whitelist and raw emission.
- uint8 matmul: not supported by bass on TRN2 (VALID_NON_TRANSPOSE_DTYPES).

Timing model (core-0 trace): graded window = [first const-pool MEMSET ~5.9us,
last teardown op]; fixed ~1.25us preamble and ~8us teardown (full 256-entry
semaphore-file reset, ~51 ops/engine) bracket the program - both compiler-
emitted and insensitive to program structure. ACT GELU ~0.83ns/elem/partition
+ 260ns/op = 1.11us per fused pair (PE-bound at 4/8 mix). Final-store DGE
completion latency ~2.3us sits on the critical path to the teardown barrier.
DGE queue startup ~2-3us: first data-dependent matmul can't start before
~10.8us; N_WARMUP_MM=8 measured optimal (7 -> HAM under-ramped, +3.7us;
9 -> delays tile 0, +1.1us). Head x DMAs: x8+x16-first-half on scalar,
weights on sync (both queues trigger ~0.7us before gpsimd's). Final state:
75.8-77.0us across runs, mean ~76.3 (baseline 84.3): head fill ~5us +
steady 8.25 tiles x ~6.7us + drain ~1.5us + store-DGE 2.3us + teardown
8.2us. Run-to-run jitter (+-1.2us) is DGE queue-startup and DMA timing
variance, not program structure.
"""

import os

import numpy as np
import ml_dtypes

import concourse.bacc as bacc
import concourse.mybir as mybir
from concourse.tile import TileContext
from concourse.bass_utils import run_bass_kernel_spmd

# Model dims (hardcoded per problem spec)
D = 512      # d_model
H = 1024     # hidden
A = 28       # max action dim
E = 4        # n embodiments
N_CORES = 8
P = 128      # partitions
TILE = 512   # main token tile
GRAIN = 128  # token granularity (min tile)
KC = D // P  # 4 contraction chunks for layer 1
HC = H // P  # 8 hidden chunks

DR_CHUNKS = 4      # h-chunks 0..DR_CHUNKS-1 use the fp8 DoubleRow path
X1_EARLY = True    # preload x tile 1 on gpsimd at head
PS_H_BUFS = 3      # fused-gelu L1 PSUM slots (2 banks each)
PS_O_BUFS = 2      # layer-2 PSUM slots (1 bank each); ps_h*2 + ps_o <= 8
N_WARMUP_MM = 8    # bridges the DMA head (~3us: enough HAM ramp);
                   # more just delays tile 0 on the in-order PE
PACK_L2 = True     # pack layer-2 into PE column groups

F32 = mybir.dt.float32
F16 = mybir.dt.float16
F8 = mybir.dt.float8e4
NP_F8 = ml_dtypes.float8_e4m3  # trn fp8e4: 4-bit exp, max normal 240

_PROGRAM_CACHE = {}

# Set by test harness to collect a profile: None | dict (filled with results)
TRACE_SINK = None


def _tile_sizes(ntok):
    sizes = [TILE] * (ntok // TILE)
    if ntok % TILE:
        sizes.append(ntok % TILE)  # remainder last: short pipeline tail
    return sizes


def _build_program(ntok, fuse_gelu, dr_hcs, descale):
    assert ntok % GRAIN == 0
    sizes = _tile_sizes(ntok)
    # fp16 x covers all 4 chunks when any h-chunk runs pure fp16, else just
    # the upper two (the DR path's fp16 half)
    kc16 = 2 if dr_hcs == HC else KC
    k0 = KC - kc16  # first chunk index covered by x16/w16
    nc = bacc.Bacc()

    # x is tile-blocked: tile t of size s occupies columns [k*off, k*(off+s))
    # as a [k, s] block, so every DMA reads contiguous per-partition runs
    x8_in = None
    if dr_hcs:
        x8_in = nc.declare_dram_parameter("x8", [P, 2 * ntok], F8,
                                          isOutput=False)
        w8_in = nc.declare_dram_parameter("w8", [P, dr_hcs, 2, P], F8,
                                          isOutput=False)
        # fp16 weights for the DR chunks' upper-dims half (memory chunk
        # order [2,3] == x16 memory chunks [0,1])
        w16d_in = nc.declare_dram_parameter("w16d", [P, dr_hcs, 2, P], F16,
                                            isOutput=False)
    x16_in = nc.declare_dram_parameter("x16", [P, kc16 * ntok], F16,
                                       isOutput=False)
    w16f_in = None
    if dr_hcs < HC:
        # full-precision chunks for the clean h-chunks, same memory chunk
        # order as x16 (KORD)
        w16f_in = nc.declare_dram_parameter(
            "w16f", [P, HC - dr_hcs, kc16, P], F16, isOutput=False)
    w2_in = nc.declare_dram_parameter("w2", [P, HC, A], F16, isOutput=False)
    b1_in = nc.declare_dram_parameter("b1", [P, HC], F32, isOutput=False)
    b2_in = nc.declare_dram_parameter("b2", [A, 1], F32, isOutput=False)
    out = nc.declare_dram_parameter("out", [A, ntok], F32, isOutput=True)

    def x_slice(x_in, k, off, size):
        a = k * off
        return x_in[:, a:a + k * size].rearrange("p (k n) -> p k n", k=k)

    with TileContext(nc) as tc:
        with (
            tc.tile_pool(name="wpool", bufs=1) as wpool,
            tc.tile_pool(name="xpool", bufs=4) as xpool,
            tc.tile_pool(name="hpool", bufs=3) as hpool,
            tc.tile_pool(name="opool", bufs=3) as opool,
            tc.tile_pool(name="ps_h", bufs=PS_H_BUFS if fuse_gelu else 6, space="PSUM") as ps_h_pool,
            tc.tile_pool(name="ps_o", bufs=PS_O_BUFS, space="PSUM") as ps_o_pool,
        ):
            # --- PE warmup: no data deps, runs during the initial DMA wait ---
            if N_WARMUP_MM:
                warm_x = wpool.tile([P, TILE], F16)
                nc.gpsimd.memset(warm_x, 0.0)
                warm_shape = [P, 2, TILE] if fuse_gelu else [P, TILE]
                warm_ps = ps_h_pool.tile(warm_shape, F32, tag="ps_h")
                warm_ps = warm_ps[:, 0] if fuse_gelu else warm_ps
                for _ in range(N_WARMUP_MM):
                    nc.tensor.matmul(warm_ps, warm_x[:, :P], warm_x,
                                     start=True, stop=True)

            # --- Weight/x loads: queues stream in parallel at kernel start
            # so the first matmuls can begin as early as possible ---
            x8_sb0 = w8_sb = w16d_sb = w16f_sb = None
            x16_sb0 = xpool.tile([P, kc16, sizes[0]], F16, tag="x16")
            if dr_hcs:
                w8_sb = wpool.tile([P, dr_hcs, 2, P], F8)
                x8_sb0 = xpool.tile([P, 2, sizes[0]], F8, tag="x8")
                nc.scalar.dma_start(out=x8_sb0,
                                    in_=x_slice(x8_in, 2, 0, sizes[0]))
                w16d_sb = wpool.tile([P, dr_hcs, 2, P], F16)
                nc.sync.dma_start(out=w8_sb, in_=w8_in[:])
                nc.sync.dma_start(out=w16d_sb, in_=w16d_in[:])
                # x16 first half rides scalar right behind x8 (scalar/sync
                # queues start ~0.7us before gpsimd's)
                nc.scalar.dma_start(
                    out=x16_sb0[:, 0:2],
                    in_=x_slice(x16_in, kc16, 0, sizes[0])[:, 0:2])
            else:
                nc.scalar.dma_start(out=x16_sb0,
                                    in_=x_slice(x16_in, kc16, 0, sizes[0]))
            if dr_hcs < HC:
                nh = HC - dr_hcs
                w16f_sb = wpool.tile([P, nh, kc16, P], F16)
                nc.sync.dma_start(out=w16f_sb[:, 0:nh // 2],
                                  in_=w16f_in[:, 0:nh // 2])
                nc.sync.dma_start(out=w16f_sb[:, nh // 2:],
                                  in_=w16f_in[:, nh // 2:])
            b1_sb = wpool.tile([P, HC], F32)
            b2_sb = wpool.tile([A, 1], F32)
            nc.gpsimd.dma_start(out=b1_sb, in_=b1_in[:])
            nc.gpsimd.dma_start(out=b2_sb, in_=b2_in[:])
            if dr_hcs and kc16 > 2:
                # second x16 half (clean chunks' lower dims, needed ~1.4us
                # into tile 0) rides scalar behind x8
                nc.scalar.dma_start(
                    out=x16_sb0[:, 2:],
                    in_=x_slice(x16_in, kc16, 0, sizes[0])[:, 2:])
            x8_sb1 = x16_sb1 = None
            if X1_EARLY and len(sizes) > 1:
                if dr_hcs:
                    x8_sb1 = xpool.tile([P, 2, sizes[1]], F8, tag="x8")
                    nc.gpsimd.dma_start(
                        out=x8_sb1, in_=x_slice(x8_in, 2, sizes[0], sizes[1]))
                x16_sb1 = xpool.tile([P, kc16, sizes[1]], F16, tag="x16")
                nc.gpsimd.dma_start(
                    out=x16_sb1, in_=x_slice(x16_in, kc16, sizes[0], sizes[1]))
            w2_sb = wpool.tile([P, HC, A], F16)
            nc.gpsimd.dma_start(out=w2_sb, in_=w2_in[:])

            def emit_l2(h_sb, off, size, t, packed):
                """Layer 2: out[:, off:off+size] = W2^T h + b2."""
                o_sb = opool.tile([A, size], F32, tag="o")
                if packed:
                    # 4 h-chunks run concurrently in the 4 PE column groups,
                    # accumulating 2 rounds; strips combined on DVE (which may
                    # read at most one PSUM operand per instruction).
                    o_ps = ps_o_pool.tile([P, size], F32, tag="ps_o")
                    for r in range(2):
                        for j in range(4):
                            hc = r * 4 + j
                            nc.tensor.matmul(
                                o_ps[32 * j:32 * j + A, :],
                                w2_sb[:, hc],
                                h_sb[:, hc],
                                start=(r == 0),
                                stop=(r == 1),
                                tile_position=(0, 32 * j),
                            )
                    nc.vector.tensor_scalar_add(o_sb, o_ps[0:A], b2_sb)
                    nc.vector.tensor_add(o_sb, o_sb, o_ps[32:32 + A])
                    nc.vector.tensor_add(o_sb, o_sb, o_ps[64:64 + A])
                    nc.vector.tensor_add(o_sb, o_sb, o_ps[96:96 + A])
                else:
                    o_ps = ps_o_pool.tile([A, size], F32, tag="ps_o")
                    for hc in range(HC):
                        nc.tensor.matmul(
                            o_ps,
                            w2_sb[:, hc],
                            h_sb[:, hc],
                            start=(hc == 0),
                            stop=(hc == HC - 1),
                        )
                    if fuse_gelu:
                        # b2 == 0: PSUM->SBUF copy on ACT (idle at the tail)
                        # so the store doesn't queue behind the previous
                        # tile's DVE strip-combine on the in-order Vector
                        nc.scalar.activation(o_sb, o_ps,
                                             mybir.ActivationFunctionType.Copy)
                    else:
                        nc.vector.tensor_scalar_add(o_sb, o_ps, b2_sb)
                # alternate store queues so the final two stores issue in
                # parallel instead of serializing on one engine
                eng = nc.sync if t % 2 == 0 else nc.scalar
                eng.dma_start(out=out[:, off:off + size], in_=o_sb)

            def emit_l1_chunk(ps, x8_sb, x16_sb, hc):
                """One h-chunk into ps [P, size]; ONE group open at a time."""
                if hc < dr_hcs:
                    nc.tensor.matmul(
                        ps, w8_sb[:, hc], x8_sb,
                        start=True, stop=False,
                        perf_mode=mybir.MatmulPerfMode.DoubleRow,
                    )
                    for i in range(2):
                        nc.tensor.matmul(
                            ps, w16d_sb[:, hc, i], x16_sb[:, i],
                            start=False, stop=(i == 1),
                        )
                else:
                    for k in range(kc16):
                        nc.tensor.matmul(
                            ps, w16f_sb[:, hc - dr_hcs, k], x16_sb[:, k],
                            start=(k == 0), stop=(k == kc16 - 1),
                        )

            # Layer 2 for tile t is emitted mid-way through layer 1 of tile
            # t+1 so its matmuls never wait on a just-finished GELU (PE is
            # in-order) and its DVE/store epilogue drains under compute. The
            # final tile uses unpacked L2: its single-op DVE epilogue keeps
            # the drain tail short.
            pend = None
            off = 0
            for t, size in enumerate(sizes):
                if t == 0:
                    x8_sb, x16_sb = x8_sb0, x16_sb0
                elif t == 1 and x16_sb1 is not None:
                    x8_sb, x16_sb = x8_sb1, x16_sb1
                else:
                    x8_sb = None
                    if dr_hcs:
                        x8_sb = xpool.tile([P, 2, size], F8, tag="x8")
                        nc.sync.dma_start(out=x8_sb,
                                          in_=x_slice(x8_in, 2, off, size))
                    x16_sb = xpool.tile([P, kc16, size], F16, tag="x16")
                    nc.sync.dma_start(out=x16_sb,
                                      in_=x_slice(x16_in, kc16, off, size))

                # --- Layer 1: h = gelu((W1^T x) * descale + b1), per chunk ---
                h_sb = hpool.tile([P, HC, size], F16, tag="h")

                def flush_pend(pend=pend):
                    if pend is not None:
                        packed = PACK_L2 and pend[3] < len(sizes) - 1
                        emit_l2(*pend, packed)

                if fuse_gelu:
                    # b1 == 0: one ACTIVATE per pair of h-chunks (2 PSUM
                    # banks) halves ACT per-op overhead; ACT is otherwise
                    # near rate-matched with PE and every hiccup stalls it.
                    for hg in range(HC // 2):
                        ps = ps_h_pool.tile([P, 2, size], F32, tag="ps_h")
                        emit_l1_chunk(ps[:, 0], x8_sb, x16_sb, hg * 2)
                        emit_l1_chunk(ps[:, 1], x8_sb, x16_sb, hg * 2 + 1)
                        nc.scalar.activation(
                            h_sb[:, hg * 2:hg * 2 + 2], ps,
                            mybir.ActivationFunctionType.Gelu,
                            scale=descale,
                        )
                        if hg == 0:
                            # previous tile's layer 2 goes here: mid-tile so
                            # its DVE/store epilogue drains before this
                            # tile's L1 ends (shorter pipeline tail)
                            flush_pend()
                else:
                    for hc in range(HC):
                        ps = ps_h_pool.tile([P, size], F32, tag="ps_h")
                        emit_l1_chunk(ps, x8_sb, x16_sb, hc)
                        nc.scalar.activation(
                            h_sb[:, hc], ps,
                            mybir.ActivationFunctionType.Gelu,
                            bias=b1_sb[:, hc:hc + 1],
                            scale=descale,
                        )
                        if hc == 1:
                            flush_pend()

                pend = (h_sb, off, size, t)
                off += size

            packed = PACK_L2 and pend[3] < len(sizes) - 1
            emit_l2(*pend, packed)

    nc.finalize()
    return nc


def _pow2_scale(absmax):
    """Largest power of two s with absmax * s <= 224 (e4m3 headroom)."""
    if absmax <= 0:
        return 1.0
    return float(2.0 ** np.floor(np.log2(224.0 / absmax)))


def kernel(pred_action_latents, W1, b1, W2, b2, embodiment_ids):
    x = np.asarray(pred_action_latents)
    W1 = np.asarray(W1)
    b1 = np.asarray(b1)
    W2 = np.asarray(W2)
    b2 = np.asarray(b2)
    ids = np.asarray(embodiment_ids)

    B, T, _ = x.shape
    assert W1.shape[0] == E and N_CORES == 2 * E

    # --- Host-side routing/sharding ---
    order = np.argsort(ids, kind="stable")
    counts = np.bincount(ids, minlength=E)
    starts = np.concatenate([[0], np.cumsum(counts)])

    # core 2e, 2e+1 handle expert e (first/second half of its rows)
    core_rows = []
    for e in range(E):
        rows_e = order[starts[e]:starts[e + 1]]
        h1 = (len(rows_e) + 1) // 2
        core_rows.append(rows_e[:h1])
        core_rows.append(rows_e[h1:])

    max_tok = max(len(r) * T for r in core_rows)
    ntok = max(GRAIN, ((max_tok + GRAIN - 1) // GRAIN) * GRAIN)

    dr_hcs = DR_CHUNKS
    kc16 = 2 if dr_hcs == HC else KC
    # x16/w16f memory chunk order: the DR chunks' fp16 half (logical contraction
    # chunks 2,3) first, so the head's split first-tile DMA lands them early
    kord = [2, 3] if kc16 == 2 else [2, 3, 0, 1]
    if dr_hcs:
        sx = _pow2_scale(np.abs(x).max())
        sw = _pow2_scale(np.abs(W1).max())
    else:
        sx = sw = 1.0
    descale = 1.0 / (sx * sw)

    fuse_gelu = not np.any(b1) and not np.any(b2)
    key = (ntok, fuse_gelu, dr_hcs, descale)
    if key not in _PROGRAM_CACHE:
        _PROGRAM_CACHE[key] = _build_program(ntok, fuse_gelu, dr_hcs, descale)
    nc = _PROGRAM_CACHE[key]

    in_maps = []
    for c in range(N_CORES):
        e = c // 2
        rows = core_rows[c]
        ntok_real = len(rows) * T
        xr = np.zeros((ntok, D), dtype=np.float32)
        xr[:ntok_real] = x[rows].reshape(ntok_real, D)
        in_map = {}
        # tile-blocked [P, k*ntok]: tile block t = [P, k, size] with
        # (p, i, n) = xr[off+n, i*P+p]; contiguous per-partition runs
        if dr_hcs:
            blocks = []
            o = 0
            for size in _tile_sizes(ntok):
                blocks.append((xr[o:o + size, :256] * sx).reshape(
                    size, 2, P).transpose(2, 1, 0).reshape(P, 2 * size))
                o += size
            in_map["x8"] = np.ascontiguousarray(
                np.concatenate(blocks, axis=1)).astype(NP_F8)
            # [P, dr_hcs, 2, P]: (p, hc, i, j) = sw*W1[e, i*P+p, hc*P+j]
            in_map["w8"] = np.ascontiguousarray(
                (W1[e, :256] * sw).reshape(2, P, HC, P)
                .transpose(1, 2, 0, 3)[:, :dr_hcs]
            ).astype(NP_F8)
        xr_k = np.stack([xr[:, c * P:(c + 1) * P] for c in kord], axis=1)
        blocks = []
        o = 0
        for size in _tile_sizes(ntok):
            blocks.append((xr_k[o:o + size] * sx).transpose(
                2, 1, 0).reshape(P, kc16 * size))
            o += size
        in_map["x16"] = np.ascontiguousarray(
            np.concatenate(blocks, axis=1)).astype(np.float16)
        # [P, nh, k, P] in kord chunk order: (p, hc, k, j) = sw*W1[e, kord[k]*P+p, hc*P+j]
        w1k = np.stack([(W1[e, c * P:(c + 1) * P] * sw) for c in kord])
        w1k = w1k.reshape(kc16, P, HC, P).transpose(1, 2, 0, 3)  # [P, HC, k, P]
        if dr_hcs:
            in_map["w16d"] = np.ascontiguousarray(
                w1k[:, :dr_hcs, 0:2]).astype(np.float16)
        if dr_hcs < HC:
            in_map["w16f"] = np.ascontiguousarray(
                w1k[:, dr_hcs:]).astype(np.float16)
        in_map["w2"] = np.ascontiguousarray(
            W2[e].reshape(HC, P, A).transpose(1, 0, 2)
        ).astype(np.float16)
        in_map["b1"] = np.ascontiguousarray(b1[e].reshape(HC, P).T).astype(np.float32)
        in_map["b2"] = np.ascontiguousarray(b2[e].reshape(A, 1)).astype(np.float32)
        in_maps.append(in_map)

    trace = TRACE_SINK is not None
    if trace:
        os.environ.pop("BASS_NEVER_TRACE", None)
    else:
        # An ambient BASS_TRACE would route run_bass_kernel_spmd through the
        # axon NTFF hook, which needs antenv.axon_hooks (absent in fresh
        # containers) — force tracing off unless explicitly requested.
        os.environ["BASS_NEVER_TRACE"] = "1"
    res = run_bass_kernel_spmd(nc, in_maps, core_ids=list(range(N_CORES)),
                               trace=trace)
    if trace:
        TRACE_SINK["exec_time_ns"] = res.exec_time_ns
        TRACE_SINK["mean_exec_time_ns"] = res.mean_exec_time_ns
        TRACE_SINK["profile_json"] = res.profile_json

    # --- Host-side unshard ---
    out_full = np.zeros((B, T, A), dtype=np.float32)
    for c in range(N_CORES):
        rows = core_rows[c]
        if len(rows) == 0:
            continue
        o = np.asarray(res.results[c]["out"])  # [A, ntok] f32
        out_full[rows] = o[:, :len(rows) * T].T.reshape(len(rows), T, A)
    return out_full
